# revision 21
# baseline (speedup 1.0000x reference)
"""DepLabeledGCN Trainium2 kernel — data-parallel variant (no collectives).

Each core processes ITS OWN batch with ALL 48 label matrices:
    s-phase:  sT[kc,l] chunks = per-label masked-adjacency matmuls (fp16,
              masks exact 0/1), label PAIRS fused into N=256 matmuls
    msum:     msg = sum_{l,kc} sT[kc,l] @ W_l^T[kc], 192 accumulating
              matmuls into one PSUM bank per layer
    relu(msg * 1/denom) -> next layer h (chunked DVE/Act ops)
then the 2-layer MLP (PE-transpose + packed PSUM) on the same core.

Weights: 24 MB fp16 streamed per label from HBM on ONE hw queue (per-core
DMA is ~410 GB/s aggregate; more queues only delays the early pairs).
The first R_RES labels stay SBUF-resident for layer 2.

Scheduling details (measured on hw traces):
  - sT tile keeps the PSUM layout [q,kc,l,i]; the psum->sbuf cast is two
    contiguous halves on vector + scalar concurrently (gpsimd cannot
    access PSUM).  msum runs l2-major so each matmul only depends on
    one label's weight DMA (layer 1 is DMA-starved; finer deps matter).
  - weight DMAs stay per-label for the same reason.
  - h0 cast and the layer-boundary relu are chunked per kc to shorten
    the critical path into each layer's first matmuls.
"""

import sys

if '/opt/trn_rl_repo' not in sys.path:
    sys.path.insert(0, '/opt/trn_rl_repo')

import numpy as np

B, N, D, L = 8, 128, 512, 48
NCORES = 8
KC = D // 128
NUM_LAYERS = 2
R_RES = 32              # labels kept resident for layer 2
NP = L // 2             # label pairs per layer

_CACHE = {}


def _build_nc():
    import concourse.bass as bass
    import concourse.mybir as mybir
    import concourse.tile as tile
    from concourse import bacc
    from concourse.masks import make_identity

    dt = mybir.dt
    f32 = dt.float32
    f16 = dt.float16
    Alu = mybir.AluOpType
    Act = mybir.ActivationFunctionType

    nc = bacc.Bacc("TRN2", target_bir_lowering=False, debug=False,
                   num_devices=NCORES)

    gcn_e = nc.dram_tensor("gcn", [N, D], f32, kind="ExternalInput").ap()
    adjT_e = nc.dram_tensor("adjT", [N, N], f32, kind="ExternalInput").ap()
    labT_e = nc.dram_tensor("labT", [N, N], f32, kind="ExternalInput").ap()
    # misc: adjR (row-major adj) + b0 + b1 packed
    misc_e = nc.dram_tensor("misc", [N, N + 2 * KC], f32,
                            kind="ExternalInput").ap()
    wT_e = nc.dram_tensor("wT", [128, L, KC, D], f16, kind="ExternalInput").ap()
    mlpw_e = nc.dram_tensor("mlpw", [128, 2, KC, D], f16,
                            kind="ExternalInput").ap()
    out_e = nc.dram_tensor("out", [128, KC, 128], f32,
                           kind="ExternalOutput").ap()

    with tile.TileContext(nc) as tc:
        with (
            tc.tile_pool(name="const", bufs=1) as cpool,
            tc.tile_pool(name="sTa", bufs=4) as sTa_pool,
            tc.tile_pool(name="sTb", bufs=4) as sTb_pool,
            tc.tile_pool(name="wst", bufs=5) as wst_pool,
            tc.tile_pool(name="spsa", bufs=3, space="PSUM") as spsa,
            tc.tile_pool(name="spsb", bufs=3, space="PSUM") as spsb,
            tc.tile_pool(name="mpsum", bufs=2, space="PSUM") as mpsum,
        ):
            # -------- critical-path input loads -----------------------------
            adjT_sb = cpool.tile([128, N], f32, tag="adjT")
            nc.sync.dma_start(adjT_sb[:], adjT_e)
            labT_sb = cpool.tile([128, N], f32, tag="labT")
            nc.sync.dma_start(labT_sb[:], labT_e)
            gcn_sb = cpool.tile([128, D], f32, tag="gcn_sb")
            nc.sync.dma_start(gcn_sb[:], gcn_e)

            h = [cpool.tile([128, D], f16, tag=f"h{ly}", name=f"h{ly}")
                 for ly in range(NUM_LAYERS + 1)]
            nc.scalar.copy(h[0][:], gcn_sb[:])

            # resident weights, loaded per label (just-in-time for layer 1)
            wres = cpool.tile([128, R_RES, KC, D], f16, tag="wres")
            for l in range(R_RES):
                nc.sync.dma_start(wres[:, l], wT_e[:, l])

            # -------- masks: maskT[j, l, i] = (labT == l) * adjT ------------
            # pairs 0..5 upfront; the rest interleaved into the layer-1 loop
            maskT = cpool.tile([128, L, N], f16, tag="maskT")

            def emit_mask(l):
                nc.vector.scalar_tensor_tensor(
                    out=maskT[:, l, :],
                    in0=labT_sb[:],
                    scalar=float(l),
                    in1=adjT_sb[:],
                    op0=Alu.is_equal,
                    op1=Alu.mult,
                )

            for l in range(12):
                emit_mask(l)

            misc_sb = cpool.tile([128, N + 2 * KC], f32, tag="misc")
            nc.sync.dma_start(misc_sb[:], misc_e)
            adjR_v = misc_sb[:, 0:N]
            b0_v = misc_sb[:, N:N + KC]
            b1_v = misc_sb[:, N + KC:N + 2 * KC]

            den = cpool.tile([128, 1], f32, tag="den")
            nc.vector.tensor_reduce(den[:], adjR_v, mybir.AxisListType.X,
                                    Alu.add)
            nc.vector.tensor_scalar_add(den[:], den[:], 1.0)
            recip = cpool.tile([128, 1], f32, tag="recip")
            nc.vector.reciprocal(recip[:], den[:])

            # identity for the MLP transposes (gpsimd, idle at start)
            identity = cpool.tile([128, 128], f16, tag="ident")
            make_identity(nc, identity[:])

            # -------- GCN layers --------------------------------------------
            def emit_s(ly, p):
                """s-phase for label pair p: one N=256 matmul per kc.
                kc 0/1 and kc 2/3 use SEPARATE psum+sbuf tiles so the
                vector and scalar psum->sbuf casts run truly in parallel
                (engines serialize on a shared psum tile)."""
                ps_a = spsa.tile([128, 2, 2, 128], f32, tag="spsa",
                                 name="spsa")
                ps_b = spsb.tile([128, 2, 2, 128], f32, tag="spsb",
                                 name="spsb")
                for kc in range(KC):
                    ps = ps_a if kc < 2 else ps_b
                    nc.tensor.matmul(
                        ps[:, kc % 2, :, :],
                        lhsT=h[ly][:, kc * 128:(kc + 1) * 128],
                        rhs=maskT[:, 2 * p:2 * p + 2, :],
                        start=True, stop=True,
                    )
                sa = sTa_pool.tile([128, 2, 2, 128], f16, tag="sTa",
                                   name="sTa")
                sb = sTb_pool.tile([128, 2, 2, 128], f16, tag="sTb",
                                   name="sTb")
                nc.vector.tensor_copy(sa[:], ps_a[:])
                nc.scalar.copy(sb[:], ps_b[:])
                return sa, sb

            def get_w(ly, p):
                """Weight pair p: resident slice or streamed tile
                (per-label DMAs keep the msum deps fine-grained)."""
                if 2 * p + 1 < R_RES:
                    return wres[:, 2 * p:2 * p + 2]
                w = wst_pool.tile([128, 2, KC, D], f16, tag="wst", name="wst")
                nc.sync.dma_start(w[:, 0], wT_e[:, 2 * p])
                nc.sync.dma_start(w[:, 1], wT_e[:, 2 * p + 1])
                return w

            S_AHEAD = 3
            for ly in range(NUM_LAYERS):
                pm = mpsum.tile([128, D], f32, tag="mm", name="mm")
                sT_q = [emit_s(ly, q) for q in range(S_AHEAD)]
                for p in range(NP):
                    if ly == 0 and 2 * (p + 6) < L:
                        emit_mask(2 * (p + 6))
                        emit_mask(2 * (p + 6) + 1)
                    if p + S_AHEAD < NP:
                        sT_q.append(emit_s(ly, p + S_AHEAD))
                    w = get_w(ly, p)
                    sa, sb = sT_q[p]
                    for l2 in range(2):
                        for kc in range(KC):
                            i = (p * 2 + l2) * KC + kc
                            st = sa if kc < 2 else sb
                            nc.tensor.matmul(
                                pm[:],
                                lhsT=st[:, kc % 2, l2, :],
                                rhs=w[:, l2, kc, :],
                                start=(i == 0), stop=(i == L * KC - 1),
                            )
                if ly == 0:
                    # MLP weights: load during layer 2 (slack window)
                    mlpw_sb = cpool.tile([128, 2, KC, D], f16, tag="mlpw")
                    nc.sync.dma_start(mlpw_sb[:], mlpw_e)
                # relu(msg * recip) -> next h (fp16), chunked per kc
                # (all on vector: engines serialize on the shared pm bank)
                for kc in range(KC):
                    sl = slice(kc * 128, (kc + 1) * 128)
                    nc.vector.tensor_scalar(h[ly + 1][:, sl], pm[:, sl],
                                            recip[:], 0.0,
                                            Alu.mult, Alu.max)

            # -------- MLP ---------------------------------------------------
            w0T_v = mlpw_sb[:, 0]
            w1T_v = mlpw_sb[:, 1]
            h_own = h[NUM_LAYERS]
            hT = cpool.tile([128, KC, 128], f16, tag="hT")
            pt = mpsum.tile([128, KC, 128], f16, tag="mm", name="ptr")
            for kc in range(KC):
                nc.tensor.transpose(pt[:, kc, :],
                                    h_own[:, kc * 128:(kc + 1) * 128],
                                    identity[:])
            nc.vector.tensor_copy(hT[:], pt[:])

            x1T = cpool.tile([128, KC, 128], f16, tag="x1T")
            px1 = mpsum.tile([128, KC, 128], f32, tag="mm", name="px1")
            for blk in range(KC):
                for kc in range(KC):
                    nc.tensor.matmul(
                        px1[:, blk, :],
                        lhsT=w0T_v[:, kc, blk * 128:(blk + 1) * 128],
                        rhs=hT[:, kc, :],
                        start=(kc == 0), stop=(kc == KC - 1),
                    )
            for blk in range(KC):
                nc.vector.tensor_scalar(x1T[:, blk, :], px1[:, blk, :],
                                        b0_v[:, blk:blk + 1], 0.0,
                                        Alu.add, Alu.max)

            x2 = cpool.tile([128, KC, 128], f32, tag="x2")
            px2 = mpsum.tile([128, KC, 128], f32, tag="mm", name="px2")
            for blk in range(KC):
                for kc in range(KC):
                    nc.tensor.matmul(
                        px2[:, blk, :],
                        lhsT=w1T_v[:, kc, blk * 128:(blk + 1) * 128],
                        rhs=x1T[:, kc, :],
                        start=(kc == 0), stop=(kc == KC - 1),
                    )
            for blk in range(KC):
                nc.vector.tensor_scalar(x2[:, blk, :], px2[:, blk, :],
                                        b1_v[:, blk:blk + 1], 0.0,
                                        Alu.add, Alu.max)

            nc.sync.dma_start(out_e, x2[:])

    nc.compile()
    return nc


def _get_nc():
    if "nc" not in _CACHE:
        _CACHE["nc"] = _build_nc()
    return _CACHE["nc"]


def kernel(gcn_inputs, word_seq_len, adj_matrix, dep_label_matrix,
           w_params, mlp_w0, mlp_b0, mlp_w1, mlp_b1, **_unused):
    from concourse.bass_utils import run_bass_kernel_spmd

    gcn = np.asarray(gcn_inputs, dtype=np.float32)
    adj = np.asarray(adj_matrix, dtype=np.float32)
    lab = np.asarray(dep_label_matrix)
    w = np.asarray(w_params, dtype=np.float32)
    w0 = np.asarray(mlp_w0, dtype=np.float32)
    w1 = np.asarray(mlp_w1, dtype=np.float32)
    b0 = np.asarray(mlp_b0, dtype=np.float32)
    b1 = np.asarray(mlp_b1, dtype=np.float32)

    # wT[kmod, l, kc, d] = w[l, d, kc*128+kmod]  (shared by all cores)
    wT = w.transpose(0, 2, 1).reshape(L, KC, 128, D).transpose(2, 0, 1, 3)
    wT = np.ascontiguousarray(wT).astype(np.float16)
    w0T = w0.T.reshape(KC, 128, D).transpose(1, 0, 2)
    w1T = w1.T.reshape(KC, 128, D).transpose(1, 0, 2)
    mlpw = np.ascontiguousarray(
        np.stack([w0T, w1T], axis=1)).astype(np.float16)   # [128, 2, KC, D]
    b0r = b0.reshape(KC, 128).T                            # [128, KC]
    b1r = b1.reshape(KC, 128).T
    labf = lab.astype(np.float32)

    in_maps = []
    for c in range(NCORES):
        miscc = np.empty((N, N + 2 * KC), dtype=np.float32)
        miscc[:, 0:N] = adj[c]
        miscc[:, N:N + KC] = b0r
        miscc[:, N + KC:N + 2 * KC] = b1r
        in_maps.append({
            "gcn": gcn[c],
            "adjT": np.ascontiguousarray(adj[c].T),
            "labT": np.ascontiguousarray(labf[c].T),
            "misc": miscc,
            "wT": wT,
            "mlpw": mlpw,
        })

    nc = _get_nc()
    res = run_bass_kernel_spmd(nc, in_maps, list(range(NCORES)))

    out = np.empty((B, N, D), dtype=np.float32)
    for c in range(NCORES):
        arr = res.results[c]["out"]          # [dmod, dblk, i]
        out[c] = np.transpose(arr, (2, 1, 0)).reshape(N, D)
    return out


# revision 31
# speedup vs baseline: 1.0426x; 1.0426x over previous
"""DepLabeledGCN Trainium2 kernel — data-parallel variant (no collectives).

Each core processes ITS OWN batch with ALL 48 label matrices:
    s-phase:  sT[kc,l] chunks = per-label masked-adjacency matmuls (fp16,
              masks exact 0/1), label PAIRS fused into N=256 matmuls
    msum:     msg = sum_{l,kc} sT[kc,l] @ W_l^T[kc], 192 accumulating
              matmuls into one PSUM bank per layer
    relu(msg * 1/denom) -> next layer h (chunked DVE/Act ops)
then the 2-layer MLP (PE-transpose + packed PSUM) on the same core.

Weights: 24 MB fp16 streamed per label from HBM on ONE hw queue (per-core
DMA is ~410 GB/s aggregate; more queues only delays the early pairs).
The first R_RES labels stay SBUF-resident for layer 2.

Scheduling details (measured on hw traces):
  - sT tile keeps the PSUM layout [q,kc,l,i]; the psum->sbuf cast is two
    contiguous halves on vector + scalar concurrently (gpsimd cannot
    access PSUM).  msum runs l2-major so each matmul only depends on
    one label's weight DMA (layer 1 is DMA-starved; finer deps matter).
  - weight DMAs stay per-label for the same reason.
  - h0 cast and the layer-boundary relu are chunked per kc to shorten
    the critical path into each layer's first matmuls.
"""

import sys

if '/opt/trn_rl_repo' not in sys.path:
    sys.path.insert(0, '/opt/trn_rl_repo')

import numpy as np

B, N, D, L = 8, 128, 512, 48
NCORES = 8
KC = D // 128
NUM_LAYERS = 2
R_RES = 32              # labels kept resident (fp16) for layer 2
NP = L // 2             # label pairs per layer
L8_LO = 16              # layer-1 labels >= L8_LO stream as e3m4 (x16)
W8_SCALE = 16.0

_CACHE = {}


def _build_nc():
    import concourse.bass as bass
    import concourse.mybir as mybir
    import concourse.tile as tile
    from concourse import bacc
    from concourse.masks import make_identity

    dt = mybir.dt
    f32 = dt.float32
    f16 = dt.float16
    Alu = mybir.AluOpType
    Act = mybir.ActivationFunctionType

    nc = bacc.Bacc("TRN2", target_bir_lowering=False, debug=False,
                   num_devices=NCORES)

    gcn_e = nc.dram_tensor("gcn", [N, D], f32, kind="ExternalInput").ap()
    adjT_e = nc.dram_tensor("adjT", [N, N], f32, kind="ExternalInput").ap()
    labT_e = nc.dram_tensor("labT", [N, N], f32, kind="ExternalInput").ap()
    # misc: adjR (row-major adj) + b0 + b1 packed
    misc_e = nc.dram_tensor("misc", [N, N + 2 * KC], f32,
                            kind="ExternalInput").ap()
    wT_e = nc.dram_tensor("wT", [128, L, KC, D], f16, kind="ExternalInput").ap()
    # layer-1 copy of labels L8_LO..L-1, e3m4 scaled x16 (half the DMA
    # bytes; the 1/16 is folded into those pairs' sT casts)
    wT8_e = nc.dram_tensor("wT8", [128, L - L8_LO, KC, D], dt.float8e3,
                           kind="ExternalInput").ap()
    mlpw_e = nc.dram_tensor("mlpw", [128, 2, KC, D], f16,
                            kind="ExternalInput").ap()
    out_e = nc.dram_tensor("out", [128, KC, 128], f32,
                           kind="ExternalOutput").ap()

    with tile.TileContext(nc) as tc:
        with (
            tc.tile_pool(name="const", bufs=1) as cpool,
            tc.tile_pool(name="sTa", bufs=4) as sTa_pool,
            tc.tile_pool(name="sTb", bufs=4) as sTb_pool,
            tc.tile_pool(name="wst", bufs=2) as wst_pool,
            tc.tile_pool(name="w8", bufs=3) as w8_pool,
            tc.tile_pool(name="spsa", bufs=3, space="PSUM") as spsa,
            tc.tile_pool(name="spsb", bufs=3, space="PSUM") as spsb,
            tc.tile_pool(name="mpsum", bufs=2, space="PSUM") as mpsum,
        ):
            # -------- critical-path input loads -----------------------------
            adjT_sb = cpool.tile([128, N], f32, tag="adjT")
            nc.sync.dma_start(adjT_sb[:], adjT_e)
            labT_sb = cpool.tile([128, N], f32, tag="labT")
            nc.sync.dma_start(labT_sb[:], labT_e)
            gcn_sb = cpool.tile([128, D], f32, tag="gcn_sb")
            nc.sync.dma_start(gcn_sb[:], gcn_e)

            h = [cpool.tile([128, D], f16, tag=f"h{ly}", name=f"h{ly}")
                 for ly in range(NUM_LAYERS + 1)]
            nc.scalar.copy(h[0][:], gcn_sb[:])

            # resident fp16 weights. Labels 0..L8_LO-1 load now (layer 1
            # consumes them JIT); labels L8_LO..R_RES-1 are only needed by
            # layer 2 and load after layer 1's e3m4 stream (queue is FIFO).
            wres = cpool.tile([128, R_RES, KC, D], f16, tag="wres")
            for l in range(L8_LO):
                nc.sync.dma_start(wres[:, l], wT_e[:, l])

            # -------- masks: maskT[j, l, i] = (labT == l) * adjT ------------
            # pairs 0..5 upfront; the rest interleaved into the layer-1 loop
            maskT = cpool.tile([128, L, N], f16, tag="maskT")

            def emit_mask(l):
                nc.vector.scalar_tensor_tensor(
                    out=maskT[:, l, :],
                    in0=labT_sb[:],
                    scalar=float(l),
                    in1=adjT_sb[:],
                    op0=Alu.is_equal,
                    op1=Alu.mult,
                )

            for l in range(12):
                emit_mask(l)

            misc_sb = cpool.tile([128, N + 2 * KC], f32, tag="misc")
            nc.sync.dma_start(misc_sb[:], misc_e)
            adjR_v = misc_sb[:, 0:N]
            b0_v = misc_sb[:, N:N + KC]
            b1_v = misc_sb[:, N + KC:N + 2 * KC]

            den = cpool.tile([128, 1], f32, tag="den")
            nc.vector.tensor_reduce(den[:], adjR_v, mybir.AxisListType.X,
                                    Alu.add)
            nc.vector.tensor_scalar_add(den[:], den[:], 1.0)
            recip = cpool.tile([128, 1], f32, tag="recip")
            nc.vector.reciprocal(recip[:], den[:])

            # identity for the MLP transposes (gpsimd, idle at start)
            identity = cpool.tile([128, 128], f16, tag="ident")
            make_identity(nc, identity[:])

            # -------- GCN layers --------------------------------------------
            def emit_s(ly, p):
                """s-phase for label pair p: one N=256 matmul per kc.
                kc 0/1 and kc 2/3 use SEPARATE psum+sbuf tiles so the
                vector and scalar psum->sbuf casts run truly in parallel
                (engines serialize on a shared psum tile).  For layer-1
                pairs whose weights stream as e3m4 (x16), the cast applies
                the exact 1/16 compensation."""
                ps_a = spsa.tile([128, 2, 2, 128], f32, tag="spsa",
                                 name="spsa")
                ps_b = spsb.tile([128, 2, 2, 128], f32, tag="spsb",
                                 name="spsb")
                for kc in range(KC):
                    ps = ps_a if kc < 2 else ps_b
                    nc.tensor.matmul(
                        ps[:, kc % 2, :, :],
                        lhsT=h[ly][:, kc * 128:(kc + 1) * 128],
                        rhs=maskT[:, 2 * p:2 * p + 2, :],
                        start=True, stop=True,
                    )
                sa = sTa_pool.tile([128, 2, 2, 128], f16, tag="sTa",
                                   name="sTa")
                sb = sTb_pool.tile([128, 2, 2, 128], f16, tag="sTb",
                                   name="sTb")
                if ly == 0 and 2 * p >= L8_LO:
                    nc.vector.tensor_scalar_mul(sa[:], ps_a[:],
                                                1.0 / W8_SCALE)
                    nc.scalar.activation(sb[:], ps_b[:], Act.Copy,
                                         scale=1.0 / W8_SCALE)
                else:
                    nc.vector.tensor_copy(sa[:], ps_a[:])
                    nc.scalar.copy(sb[:], ps_b[:])
                return sa, sb

            def get_w(ly, p):
                """Weight pair p.  Layer 1: resident fp16 for labels
                < L8_LO, else an e3m4 stream tile.  Layer 2: resident fp16
                or a re-streamed fp16 tile.  Per-label DMAs keep the msum
                deps fine-grained."""
                if ly == 0 and 2 * p >= L8_LO:
                    w = w8_pool.tile([128, 2, KC, D], dt.float8e3, tag="w8",
                                     name="w8")
                    nc.sync.dma_start(w[:, 0], wT8_e[:, 2 * p - L8_LO])
                    nc.sync.dma_start(w[:, 1], wT8_e[:, 2 * p + 1 - L8_LO])
                    return w
                if 2 * p + 1 < R_RES:
                    return wres[:, 2 * p:2 * p + 2]
                w = wst_pool.tile([128, 2, KC, D], f16, tag="wst", name="wst")
                nc.sync.dma_start(w[:, 0], wT_e[:, 2 * p])
                nc.sync.dma_start(w[:, 1], wT_e[:, 2 * p + 1])
                return w

            S_AHEAD = 2
            for ly in range(NUM_LAYERS):
                pm = mpsum.tile([128, D], f32, tag="mm", name="mm")
                sT_q = [emit_s(ly, q) for q in range(S_AHEAD)]
                for p in range(NP):
                    if ly == 0 and 2 * (p + 6) < L:
                        emit_mask(2 * (p + 6))
                        emit_mask(2 * (p + 6) + 1)
                    if p + S_AHEAD < NP:
                        sT_q.append(emit_s(ly, p + S_AHEAD))
                    w = get_w(ly, p)
                    sa, sb = sT_q[p]
                    for l2 in range(2):
                        for kc in range(KC):
                            i = (p * 2 + l2) * KC + kc
                            st = sa if kc < 2 else sb
                            nc.tensor.matmul(
                                pm[:],
                                lhsT=st[:, kc % 2, l2, :],
                                rhs=w[:, l2, kc, :],
                                start=(i == 0), stop=(i == L * KC - 1),
                            )
                if ly == 0:
                    # rest of the fp16 residents (layer-2 only) + MLP
                    # weights: queue them behind layer 1's e3m4 stream
                    for l in range(L8_LO, R_RES):
                        nc.sync.dma_start(wres[:, l], wT_e[:, l])
                    mlpw_sb = cpool.tile([128, 2, KC, D], f16, tag="mlpw")
                    nc.sync.dma_start(mlpw_sb[:], mlpw_e)
                # relu(msg * recip) -> next h (fp16), chunked per kc
                # (all on vector: engines serialize on the shared pm bank)
                for kc in range(KC):
                    sl = slice(kc * 128, (kc + 1) * 128)
                    nc.vector.tensor_scalar(h[ly + 1][:, sl], pm[:, sl],
                                            recip[:], 0.0,
                                            Alu.mult, Alu.max)

            # -------- MLP ---------------------------------------------------
            w0T_v = mlpw_sb[:, 0]
            w1T_v = mlpw_sb[:, 1]
            h_own = h[NUM_LAYERS]
            hT = cpool.tile([128, KC, 128], f16, tag="hT")
            pt = mpsum.tile([128, KC, 128], f16, tag="mm", name="ptr")
            for kc in range(KC):
                nc.tensor.transpose(pt[:, kc, :],
                                    h_own[:, kc * 128:(kc + 1) * 128],
                                    identity[:])
            nc.vector.tensor_copy(hT[:], pt[:])

            x1T = cpool.tile([128, KC, 128], f16, tag="x1T")
            px1 = mpsum.tile([128, KC, 128], f32, tag="mm", name="px1")
            for blk in range(KC):
                for kc in range(KC):
                    nc.tensor.matmul(
                        px1[:, blk, :],
                        lhsT=w0T_v[:, kc, blk * 128:(blk + 1) * 128],
                        rhs=hT[:, kc, :],
                        start=(kc == 0), stop=(kc == KC - 1),
                    )
            for blk in range(KC):
                nc.vector.tensor_scalar(x1T[:, blk, :], px1[:, blk, :],
                                        b0_v[:, blk:blk + 1], 0.0,
                                        Alu.add, Alu.max)

            x2 = cpool.tile([128, KC, 128], f32, tag="x2")
            px2 = mpsum.tile([128, KC, 128], f32, tag="mm", name="px2")
            for blk in range(KC):
                for kc in range(KC):
                    nc.tensor.matmul(
                        px2[:, blk, :],
                        lhsT=w1T_v[:, kc, blk * 128:(blk + 1) * 128],
                        rhs=x1T[:, kc, :],
                        start=(kc == 0), stop=(kc == KC - 1),
                    )
            for blk in range(KC):
                nc.vector.tensor_scalar(x2[:, blk, :], px2[:, blk, :],
                                        b1_v[:, blk:blk + 1], 0.0,
                                        Alu.add, Alu.max)

            nc.sync.dma_start(out_e, x2[:])

    nc.compile()
    return nc


def _get_nc():
    if "nc" not in _CACHE:
        _CACHE["nc"] = _build_nc()
    return _CACHE["nc"]


def kernel(gcn_inputs, word_seq_len, adj_matrix, dep_label_matrix,
           w_params, mlp_w0, mlp_b0, mlp_w1, mlp_b1, **_unused):
    from concourse.bass_utils import run_bass_kernel_spmd

    gcn = np.asarray(gcn_inputs, dtype=np.float32)
    adj = np.asarray(adj_matrix, dtype=np.float32)
    lab = np.asarray(dep_label_matrix)
    w = np.asarray(w_params, dtype=np.float32)
    w0 = np.asarray(mlp_w0, dtype=np.float32)
    w1 = np.asarray(mlp_w1, dtype=np.float32)
    b0 = np.asarray(mlp_b0, dtype=np.float32)
    b1 = np.asarray(mlp_b1, dtype=np.float32)

    import ml_dtypes

    # wT[kmod, l, kc, d] = w[l, d, kc*128+kmod]  (shared by all cores)
    wT32 = w.transpose(0, 2, 1).reshape(L, KC, 128, D).transpose(2, 0, 1, 3)
    wT32 = np.ascontiguousarray(wT32)
    wT = wT32.astype(np.float16)
    # layer-1 e3m4 copy of labels L8_LO.., scaled x16 to clear denormals
    wT8 = np.ascontiguousarray(
        (wT32[:, L8_LO:] * W8_SCALE)).astype(ml_dtypes.float8_e3m4)
    w0T = w0.T.reshape(KC, 128, D).transpose(1, 0, 2)
    w1T = w1.T.reshape(KC, 128, D).transpose(1, 0, 2)
    mlpw = np.ascontiguousarray(
        np.stack([w0T, w1T], axis=1)).astype(np.float16)   # [128, 2, KC, D]
    b0r = b0.reshape(KC, 128).T                            # [128, KC]
    b1r = b1.reshape(KC, 128).T
    labf = lab.astype(np.float32)

    in_maps = []
    for c in range(NCORES):
        miscc = np.empty((N, N + 2 * KC), dtype=np.float32)
        miscc[:, 0:N] = adj[c]
        miscc[:, N:N + KC] = b0r
        miscc[:, N + KC:N + 2 * KC] = b1r
        in_maps.append({
            "gcn": gcn[c],
            "adjT": np.ascontiguousarray(adj[c].T),
            "labT": np.ascontiguousarray(labf[c].T),
            "misc": miscc,
            "wT": wT,
            "wT8": wT8,
            "mlpw": mlpw,
        })

    nc = _get_nc()
    res = run_bass_kernel_spmd(nc, in_maps, list(range(NCORES)))

    out = np.empty((B, N, D), dtype=np.float32)
    for c in range(NCORES):
        arr = res.results[c]["out"]          # [dmod, dblk, i]
        out[c] = np.transpose(arr, (2, 1, 0)).reshape(N, D)
    return out


# revision 33
# speedup vs baseline: 1.0863x; 1.0419x over previous
"""DepLabeledGCN Trainium2 kernel — data-parallel variant (no collectives).

Each core processes ITS OWN batch with ALL 48 label matrices:
    s-phase:  sT[kc,l] chunks = per-label masked-adjacency matmuls (fp16,
              masks exact 0/1), label PAIRS fused into N=256 matmuls
    msum:     msg = sum_{l,kc} sT[kc,l] @ W_l^T[kc], 192 accumulating
              matmuls into one PSUM bank per layer
    relu(msg * 1/denom) -> next layer h (chunked DVE/Act ops)
then the 2-layer MLP (PE-transpose + packed PSUM) on the same core.

Weights: 24 MB fp16 streamed per label from HBM on ONE hw queue (per-core
DMA is ~410 GB/s aggregate; more queues only delays the early pairs).
The first R_RES labels stay SBUF-resident for layer 2.

Scheduling details (measured on hw traces):
  - sT tile keeps the PSUM layout [q,kc,l,i]; the psum->sbuf cast is two
    contiguous halves on vector + scalar concurrently (gpsimd cannot
    access PSUM).  msum runs l2-major so each matmul only depends on
    one label's weight DMA (layer 1 is DMA-starved; finer deps matter).
  - weight DMAs stay per-label for the same reason.
  - h0 cast and the layer-boundary relu are chunked per kc to shorten
    the critical path into each layer's first matmuls.
"""

import sys

if '/opt/trn_rl_repo' not in sys.path:
    sys.path.insert(0, '/opt/trn_rl_repo')

import numpy as np

B, N, D, L = 8, 128, 512, 48
NCORES = 8
KC = D // 128
NUM_LAYERS = 2
R_RES = 28              # labels kept resident (fp16) for layer 2
NP = L // 2             # label pairs per layer
L8_LO = 16              # layer-1 labels >= L8_LO stream as e3m4 (x16)
W8_SCALE = 16.0

_CACHE = {}


def _build_nc():
    import concourse.bass as bass
    import concourse.mybir as mybir
    import concourse.tile as tile
    from concourse import bacc
    from concourse.masks import make_identity

    dt = mybir.dt
    f32 = dt.float32
    f16 = dt.float16
    Alu = mybir.AluOpType
    Act = mybir.ActivationFunctionType

    nc = bacc.Bacc("TRN2", target_bir_lowering=False, debug=False,
                   num_devices=NCORES)

    gcn_e = nc.dram_tensor("gcn", [N, D], f32, kind="ExternalInput").ap()
    adjT_e = nc.dram_tensor("adjT", [N, N], f32, kind="ExternalInput").ap()
    labT_e = nc.dram_tensor("labT", [N, N], f32, kind="ExternalInput").ap()
    # misc: adjR (row-major adj) + b0 + b1 packed
    misc_e = nc.dram_tensor("misc", [N, N + 2 * KC], f32,
                            kind="ExternalInput").ap()
    wT_e = nc.dram_tensor("wT", [128, L, KC, D], f16, kind="ExternalInput").ap()
    # layer-1 copy of labels L8_LO..L-1, e3m4 scaled x16 (half the DMA
    # bytes; the 1/16 is folded into those pairs' sT casts)
    wT8_e = nc.dram_tensor("wT8", [128, L - L8_LO, KC, D], dt.float8e3,
                           kind="ExternalInput").ap()
    mlpw_e = nc.dram_tensor("mlpw", [128, 2, KC, D], f16,
                            kind="ExternalInput").ap()
    out_e = nc.dram_tensor("out", [128, KC, 128], f32,
                           kind="ExternalOutput").ap()

    with tile.TileContext(nc) as tc:
        with (
            tc.tile_pool(name="const", bufs=1) as cpool,
            tc.tile_pool(name="sTa", bufs=4) as sTa_pool,
            tc.tile_pool(name="sTb", bufs=4) as sTb_pool,
            tc.tile_pool(name="wst", bufs=4) as wst_pool,
            tc.tile_pool(name="w8", bufs=4) as w8_pool,
            tc.tile_pool(name="spsa", bufs=3, space="PSUM") as spsa,
            tc.tile_pool(name="spsb", bufs=3, space="PSUM") as spsb,
            tc.tile_pool(name="mpsum", bufs=2, space="PSUM") as mpsum,
        ):
            # -------- critical-path input loads -----------------------------
            adjT_sb = cpool.tile([128, N], f32, tag="adjT")
            nc.sync.dma_start(adjT_sb[:], adjT_e)
            labT_sb = cpool.tile([128, N], f32, tag="labT")
            nc.sync.dma_start(labT_sb[:], labT_e)
            gcn_sb = cpool.tile([128, D], f32, tag="gcn_sb")
            nc.sync.dma_start(gcn_sb[:], gcn_e)

            h = [cpool.tile([128, D], f16, tag=f"h{ly}", name=f"h{ly}")
                 for ly in range(NUM_LAYERS + 1)]
            nc.scalar.copy(h[0][:], gcn_sb[:])

            # resident fp16 weights. Labels 0..L8_LO-1 load now (layer 1
            # consumes them JIT); labels L8_LO..R_RES-1 are only needed by
            # layer 2 and load after layer 1's e3m4 stream (queue is FIFO).
            wres = cpool.tile([128, R_RES, KC, D], f16, tag="wres")
            for l in range(L8_LO):
                nc.sync.dma_start(wres[:, l], wT_e[:, l])

            # -------- masks: maskT[j, l, i] = (labT == l) * adjT ------------
            # pairs 0..5 upfront; the rest interleaved into the layer-1 loop
            maskT = cpool.tile([128, L, N], f16, tag="maskT")

            def emit_mask(l):
                nc.vector.scalar_tensor_tensor(
                    out=maskT[:, l, :],
                    in0=labT_sb[:],
                    scalar=float(l),
                    in1=adjT_sb[:],
                    op0=Alu.is_equal,
                    op1=Alu.mult,
                )

            for l in range(12):
                emit_mask(l)

            misc_sb = cpool.tile([128, N + 2 * KC], f32, tag="misc")
            nc.sync.dma_start(misc_sb[:], misc_e)
            adjR_v = misc_sb[:, 0:N]
            b0_v = misc_sb[:, N:N + KC]
            b1_v = misc_sb[:, N + KC:N + 2 * KC]

            den = cpool.tile([128, 1], f32, tag="den")
            nc.vector.tensor_reduce(den[:], adjR_v, mybir.AxisListType.X,
                                    Alu.add)
            nc.vector.tensor_scalar_add(den[:], den[:], 1.0)
            recip = cpool.tile([128, 1], f32, tag="recip")
            nc.vector.reciprocal(recip[:], den[:])

            # identity for the MLP transposes (gpsimd, idle at start)
            identity = cpool.tile([128, 128], f16, tag="ident")
            make_identity(nc, identity[:])

            # -------- GCN layers --------------------------------------------
            def emit_s(ly, p):
                """s-phase for label pair p: one N=256 matmul per kc.
                kc 0/1 and kc 2/3 use SEPARATE psum+sbuf tiles so the
                vector and scalar psum->sbuf casts run truly in parallel
                (engines serialize on a shared psum tile).  For layer-1
                pairs whose weights stream as e3m4 (x16), the cast applies
                the exact 1/16 compensation."""
                ps_a = spsa.tile([128, 2, 2, 128], f32, tag="spsa",
                                 name="spsa")
                ps_b = spsb.tile([128, 2, 2, 128], f32, tag="spsb",
                                 name="spsb")
                for kc in range(KC):
                    ps = ps_a if kc < 2 else ps_b
                    nc.tensor.matmul(
                        ps[:, kc % 2, :, :],
                        lhsT=h[ly][:, kc * 128:(kc + 1) * 128],
                        rhs=maskT[:, 2 * p:2 * p + 2, :],
                        start=True, stop=True,
                    )
                sa = sTa_pool.tile([128, 2, 2, 128], f16, tag="sTa",
                                   name="sTa")
                sb = sTb_pool.tile([128, 2, 2, 128], f16, tag="sTb",
                                   name="sTb")
                if ly == 0 and 2 * p >= L8_LO:
                    nc.vector.tensor_scalar_mul(sa[:], ps_a[:],
                                                1.0 / W8_SCALE)
                    nc.scalar.activation(sb[:], ps_b[:], Act.Copy,
                                         scale=1.0 / W8_SCALE)
                else:
                    nc.vector.tensor_copy(sa[:], ps_a[:])
                    nc.scalar.copy(sb[:], ps_b[:])
                return sa, sb

            def get_w(ly, p):
                """Weight pair p.  Layer 1: resident fp16 for labels
                < L8_LO, else an e3m4 stream tile.  Layer 2: resident fp16
                or a re-streamed fp16 tile.  Per-label DMAs keep the msum
                deps fine-grained."""
                if ly == 0 and 2 * p >= L8_LO:
                    w = w8_pool.tile([128, 2, KC, D], dt.float8e3, tag="w8",
                                     name="w8")
                    nc.sync.dma_start(w[:, 0], wT8_e[:, 2 * p - L8_LO])
                    nc.sync.dma_start(w[:, 1], wT8_e[:, 2 * p + 1 - L8_LO])
                    return w
                if 2 * p + 1 < R_RES:
                    return wres[:, 2 * p:2 * p + 2]
                w = wst_pool.tile([128, 2, KC, D], f16, tag="wst", name="wst")
                nc.sync.dma_start(w[:, 0], wT_e[:, 2 * p])
                nc.sync.dma_start(w[:, 1], wT_e[:, 2 * p + 1])
                return w

            S_AHEAD = 2
            for ly in range(NUM_LAYERS):
                pm = mpsum.tile([128, D], f32, tag="mm", name="mm")
                sT_q = [emit_s(ly, q) for q in range(S_AHEAD)]
                for p in range(NP):
                    if ly == 0 and 2 * (p + 6) < L:
                        emit_mask(2 * (p + 6))
                        emit_mask(2 * (p + 6) + 1)
                    if p + S_AHEAD < NP:
                        sT_q.append(emit_s(ly, p + S_AHEAD))
                    w = get_w(ly, p)
                    sa, sb = sT_q[p]
                    for l2 in range(2):
                        for kc in range(KC):
                            i = (p * 2 + l2) * KC + kc
                            st = sa if kc < 2 else sb
                            nc.tensor.matmul(
                                pm[:],
                                lhsT=st[:, kc % 2, l2, :],
                                rhs=w[:, l2, kc, :],
                                start=(i == 0), stop=(i == L * KC - 1),
                            )
                if ly == 0:
                    # rest of the fp16 residents (layer-2 only) + MLP
                    # weights: queue them behind layer 1's e3m4 stream
                    for l in range(L8_LO, R_RES):
                        nc.sync.dma_start(wres[:, l], wT_e[:, l])
                    mlpw_sb = cpool.tile([128, 2, KC, D], f16, tag="mlpw")
                    nc.sync.dma_start(mlpw_sb[:], mlpw_e)
                # relu(msg * recip) -> next h (fp16), chunked per kc
                # (all on vector: engines serialize on the shared pm bank)
                for kc in range(KC):
                    sl = slice(kc * 128, (kc + 1) * 128)
                    nc.vector.tensor_scalar(h[ly + 1][:, sl], pm[:, sl],
                                            recip[:], 0.0,
                                            Alu.mult, Alu.max)

            # -------- MLP ---------------------------------------------------
            w0T_v = mlpw_sb[:, 0]
            w1T_v = mlpw_sb[:, 1]
            h_own = h[NUM_LAYERS]
            hT = cpool.tile([128, KC, 128], f16, tag="hT")
            pt = mpsum.tile([128, KC, 128], f16, tag="mm", name="ptr")
            for kc in range(KC):
                nc.tensor.transpose(pt[:, kc, :],
                                    h_own[:, kc * 128:(kc + 1) * 128],
                                    identity[:])
            nc.vector.tensor_copy(hT[:], pt[:])

            x1T = cpool.tile([128, KC, 128], f16, tag="x1T")
            px1 = mpsum.tile([128, KC, 128], f32, tag="mm", name="px1")
            for blk in range(KC):
                for kc in range(KC):
                    nc.tensor.matmul(
                        px1[:, blk, :],
                        lhsT=w0T_v[:, kc, blk * 128:(blk + 1) * 128],
                        rhs=hT[:, kc, :],
                        start=(kc == 0), stop=(kc == KC - 1),
                    )
            for blk in range(KC):
                nc.vector.tensor_scalar(x1T[:, blk, :], px1[:, blk, :],
                                        b0_v[:, blk:blk + 1], 0.0,
                                        Alu.add, Alu.max)

            x2 = cpool.tile([128, KC, 128], f32, tag="x2")
            px2 = mpsum.tile([128, KC, 128], f32, tag="mm", name="px2")
            for blk in range(KC):
                for kc in range(KC):
                    nc.tensor.matmul(
                        px2[:, blk, :],
                        lhsT=w1T_v[:, kc, blk * 128:(blk + 1) * 128],
                        rhs=x1T[:, kc, :],
                        start=(kc == 0), stop=(kc == KC - 1),
                    )
            for blk in range(KC):
                nc.vector.tensor_scalar(x2[:, blk, :], px2[:, blk, :],
                                        b1_v[:, blk:blk + 1], 0.0,
                                        Alu.add, Alu.max)

            nc.sync.dma_start(out_e, x2[:])

    nc.compile()
    return nc


def _get_nc():
    if "nc" not in _CACHE:
        _CACHE["nc"] = _build_nc()
    return _CACHE["nc"]


def kernel(gcn_inputs, word_seq_len, adj_matrix, dep_label_matrix,
           w_params, mlp_w0, mlp_b0, mlp_w1, mlp_b1, **_unused):
    from concourse.bass_utils import run_bass_kernel_spmd

    gcn = np.asarray(gcn_inputs, dtype=np.float32)
    adj = np.asarray(adj_matrix, dtype=np.float32)
    lab = np.asarray(dep_label_matrix)
    w = np.asarray(w_params, dtype=np.float32)
    w0 = np.asarray(mlp_w0, dtype=np.float32)
    w1 = np.asarray(mlp_w1, dtype=np.float32)
    b0 = np.asarray(mlp_b0, dtype=np.float32)
    b1 = np.asarray(mlp_b1, dtype=np.float32)

    import ml_dtypes

    # wT[kmod, l, kc, d] = w[l, d, kc*128+kmod]  (shared by all cores)
    wT32 = w.transpose(0, 2, 1).reshape(L, KC, 128, D).transpose(2, 0, 1, 3)
    wT32 = np.ascontiguousarray(wT32)
    wT = wT32.astype(np.float16)
    # layer-1 e3m4 copy of labels L8_LO.., scaled x16 to clear denormals
    wT8 = np.ascontiguousarray(
        (wT32[:, L8_LO:] * W8_SCALE)).astype(ml_dtypes.float8_e3m4)
    w0T = w0.T.reshape(KC, 128, D).transpose(1, 0, 2)
    w1T = w1.T.reshape(KC, 128, D).transpose(1, 0, 2)
    mlpw = np.ascontiguousarray(
        np.stack([w0T, w1T], axis=1)).astype(np.float16)   # [128, 2, KC, D]
    b0r = b0.reshape(KC, 128).T                            # [128, KC]
    b1r = b1.reshape(KC, 128).T
    labf = lab.astype(np.float32)

    in_maps = []
    for c in range(NCORES):
        miscc = np.empty((N, N + 2 * KC), dtype=np.float32)
        miscc[:, 0:N] = adj[c]
        miscc[:, N:N + KC] = b0r
        miscc[:, N + KC:N + 2 * KC] = b1r
        in_maps.append({
            "gcn": gcn[c],
            "adjT": np.ascontiguousarray(adj[c].T),
            "labT": np.ascontiguousarray(labf[c].T),
            "misc": miscc,
            "wT": wT,
            "wT8": wT8,
            "mlpw": mlpw,
        })

    nc = _get_nc()
    res = run_bass_kernel_spmd(nc, in_maps, list(range(NCORES)))

    out = np.empty((B, N, D), dtype=np.float32)
    for c in range(NCORES):
        arr = res.results[c]["out"]          # [dmod, dblk, i]
        out[c] = np.transpose(arr, (2, 1, 0)).reshape(N, D)
    return out


# revision 36
# speedup vs baseline: 1.1424x; 1.0517x over previous
"""DepLabeledGCN Trainium2 kernel — data-parallel variant (no collectives).

Each core processes ITS OWN batch with ALL 48 label matrices:
    s-phase:  sT[kc,l] chunks = per-label masked-adjacency matmuls (fp16,
              masks exact 0/1), label PAIRS fused into N=256 matmuls
    msum:     msg = sum_{l,kc} sT[kc,l] @ W_l^T[kc], 192 accumulating
              matmuls into one PSUM bank per layer
    relu(msg * 1/denom) -> next layer h (chunked DVE/Act ops)
then the 2-layer MLP (PE-transpose + packed PSUM) on the same core.

Weights: 24 MB fp16 streamed per label from HBM on ONE hw queue (per-core
DMA is ~410 GB/s aggregate; more queues only delays the early pairs).
The first R_RES labels stay SBUF-resident for layer 2.

Scheduling details (measured on hw traces):
  - sT tile keeps the PSUM layout [q,kc,l,i]; the psum->sbuf cast is two
    contiguous halves on vector + scalar concurrently (gpsimd cannot
    access PSUM).  msum runs l2-major so each matmul only depends on
    one label's weight DMA (layer 1 is DMA-starved; finer deps matter).
  - weight DMAs stay per-label for the same reason.
  - h0 cast and the layer-boundary relu are chunked per kc to shorten
    the critical path into each layer's first matmuls.
"""

import sys

if '/opt/trn_rl_repo' not in sys.path:
    sys.path.insert(0, '/opt/trn_rl_repo')

import numpy as np

B, N, D, L = 8, 128, 512, 48
NCORES = 8
KC = D // 128
NUM_LAYERS = 2
R_RES = 28              # labels kept resident (fp16) for layer 2
NP = L // 2             # label pairs per layer
L8_LO = 8               # layer-1 labels >= L8_LO stream as e3m4 (x16)
L2_8LO = 36             # layer-2 labels >= L2_8LO stream as e3m4 (x16)
W8_SCALE = 16.0


def _use8(ly, p):
    return (ly == 0 and 2 * p >= L8_LO) or (ly == 1 and 2 * p >= L2_8LO)

_CACHE = {}


def _build_nc():
    import concourse.bass as bass
    import concourse.mybir as mybir
    import concourse.tile as tile
    from concourse import bacc
    from concourse.masks import make_identity

    dt = mybir.dt
    f32 = dt.float32
    f16 = dt.float16
    Alu = mybir.AluOpType
    Act = mybir.ActivationFunctionType

    nc = bacc.Bacc("TRN2", target_bir_lowering=False, debug=False,
                   num_devices=NCORES)

    gcn_e = nc.dram_tensor("gcn", [N, D], f32, kind="ExternalInput").ap()
    adjT_e = nc.dram_tensor("adjT", [N, N], f32, kind="ExternalInput").ap()
    labT_e = nc.dram_tensor("labT", [N, N], f32, kind="ExternalInput").ap()
    # misc: adjR (row-major adj) + b0 + b1 packed
    misc_e = nc.dram_tensor("misc", [N, N + 2 * KC], f32,
                            kind="ExternalInput").ap()
    wT_e = nc.dram_tensor("wT", [128, L, KC, D], f16, kind="ExternalInput").ap()
    # layer-1 copy of labels L8_LO..L-1, e3m4 scaled x16 (half the DMA
    # bytes; the 1/16 is folded into those pairs' sT casts)
    wT8_e = nc.dram_tensor("wT8", [128, L - L8_LO, KC, D], dt.float8e3,
                           kind="ExternalInput").ap()
    mlpw_e = nc.dram_tensor("mlpw", [128, 2, KC, D], f16,
                            kind="ExternalInput").ap()
    out_e = nc.dram_tensor("out", [128, KC, 128], f32,
                           kind="ExternalOutput").ap()

    with tile.TileContext(nc) as tc:
        with (
            tc.tile_pool(name="const", bufs=1) as cpool,
            tc.tile_pool(name="sTa", bufs=4) as sTa_pool,
            tc.tile_pool(name="sTb", bufs=4) as sTb_pool,
            tc.tile_pool(name="wst", bufs=4) as wst_pool,
            tc.tile_pool(name="w8", bufs=4) as w8_pool,
            tc.tile_pool(name="spsa", bufs=3, space="PSUM") as spsa,
            tc.tile_pool(name="spsb", bufs=3, space="PSUM") as spsb,
            tc.tile_pool(name="mpsum", bufs=2, space="PSUM") as mpsum,
        ):
            # -------- critical-path input loads -----------------------------
            adjT_sb = cpool.tile([128, N], f32, tag="adjT")
            nc.sync.dma_start(adjT_sb[:], adjT_e)
            labT_sb = cpool.tile([128, N], f32, tag="labT")
            nc.sync.dma_start(labT_sb[:], labT_e)
            gcn_sb = cpool.tile([128, D], f32, tag="gcn_sb")
            nc.sync.dma_start(gcn_sb[:], gcn_e)

            h = [cpool.tile([128, D], f16, tag=f"h{ly}", name=f"h{ly}")
                 for ly in range(NUM_LAYERS + 1)]
            nc.scalar.copy(h[0][:], gcn_sb[:])

            # resident fp16 weights. Labels 0..L8_LO-1 load now (layer 1
            # consumes them JIT); labels L8_LO..R_RES-1 are only needed by
            # layer 2 and load after layer 1's e3m4 stream (queue is FIFO).
            wres = cpool.tile([128, R_RES, KC, D], f16, tag="wres")
            for l in range(L8_LO):
                nc.sync.dma_start(wres[:, l], wT_e[:, l])

            # -------- masks: maskT[j, l, i] = (labT == l) * adjT ------------
            # pairs 0..5 upfront; the rest interleaved into the layer-1 loop
            maskT = cpool.tile([128, L, N], f16, tag="maskT")

            def emit_mask(l):
                nc.vector.scalar_tensor_tensor(
                    out=maskT[:, l, :],
                    in0=labT_sb[:],
                    scalar=float(l),
                    in1=adjT_sb[:],
                    op0=Alu.is_equal,
                    op1=Alu.mult,
                )

            for l in range(12):
                emit_mask(l)

            misc_sb = cpool.tile([128, N + 2 * KC], f32, tag="misc")
            nc.sync.dma_start(misc_sb[:], misc_e)
            adjR_v = misc_sb[:, 0:N]
            b0_v = misc_sb[:, N:N + KC]
            b1_v = misc_sb[:, N + KC:N + 2 * KC]

            den = cpool.tile([128, 1], f32, tag="den")
            nc.vector.tensor_reduce(den[:], adjR_v, mybir.AxisListType.X,
                                    Alu.add)
            nc.vector.tensor_scalar_add(den[:], den[:], 1.0)
            recip = cpool.tile([128, 1], f32, tag="recip")
            nc.vector.reciprocal(recip[:], den[:])

            # identity for the MLP transposes (gpsimd, idle at start)
            identity = cpool.tile([128, 128], f16, tag="ident")
            make_identity(nc, identity[:])

            # -------- GCN layers --------------------------------------------
            def emit_s(ly, p):
                """s-phase for label pair p: one N=256 matmul per kc.
                kc 0/1 and kc 2/3 use SEPARATE psum+sbuf tiles so the
                vector and scalar psum->sbuf casts run truly in parallel
                (engines serialize on a shared psum tile).  For layer-1
                pairs whose weights stream as e3m4 (x16), the cast applies
                the exact 1/16 compensation."""
                ps_a = spsa.tile([128, 2, 2, 128], f32, tag="spsa",
                                 name="spsa")
                ps_b = spsb.tile([128, 2, 2, 128], f32, tag="spsb",
                                 name="spsb")
                for kc in range(KC):
                    ps = ps_a if kc < 2 else ps_b
                    nc.tensor.matmul(
                        ps[:, kc % 2, :, :],
                        lhsT=h[ly][:, kc * 128:(kc + 1) * 128],
                        rhs=maskT[:, 2 * p:2 * p + 2, :],
                        start=True, stop=True,
                    )
                sa = sTa_pool.tile([128, 2, 2, 128], f16, tag="sTa",
                                   name="sTa")
                sb = sTb_pool.tile([128, 2, 2, 128], f16, tag="sTb",
                                   name="sTb")
                if _use8(ly, p):
                    nc.vector.tensor_scalar_mul(sa[:], ps_a[:],
                                                1.0 / W8_SCALE)
                    nc.scalar.activation(sb[:], ps_b[:], Act.Copy,
                                         scale=1.0 / W8_SCALE)
                else:
                    nc.vector.tensor_copy(sa[:], ps_a[:])
                    nc.scalar.copy(sb[:], ps_b[:])
                return sa, sb

            def get_w(ly, p):
                """Weight pair p.  Layer 1: resident fp16 for labels
                < L8_LO, else an e3m4 stream tile.  Layer 2: resident fp16
                or a re-streamed fp16 tile.  Per-label DMAs keep the msum
                deps fine-grained."""
                if _use8(ly, p):
                    w = w8_pool.tile([128, 2, KC, D], dt.float8e3, tag="w8",
                                     name="w8")
                    nc.sync.dma_start(w[:, 0], wT8_e[:, 2 * p - L8_LO])
                    nc.sync.dma_start(w[:, 1], wT8_e[:, 2 * p + 1 - L8_LO])
                    return w
                if 2 * p + 1 < R_RES:
                    return wres[:, 2 * p:2 * p + 2]
                w = wst_pool.tile([128, 2, KC, D], f16, tag="wst", name="wst")
                nc.sync.dma_start(w[:, 0], wT_e[:, 2 * p])
                nc.sync.dma_start(w[:, 1], wT_e[:, 2 * p + 1])
                return w

            S_AHEAD = 2
            for ly in range(NUM_LAYERS):
                pm = mpsum.tile([128, D], f32, tag="mm", name="mm")
                sT_q = [emit_s(ly, q) for q in range(S_AHEAD)]
                for p in range(NP):
                    if ly == 0 and 2 * (p + 6) < L:
                        emit_mask(2 * (p + 6))
                        emit_mask(2 * (p + 6) + 1)
                    if p + S_AHEAD < NP:
                        sT_q.append(emit_s(ly, p + S_AHEAD))
                    w = get_w(ly, p)
                    sa, sb = sT_q[p]
                    for l2 in range(2):
                        for kc in range(KC):
                            i = (p * 2 + l2) * KC + kc
                            st = sa if kc < 2 else sb
                            nc.tensor.matmul(
                                pm[:],
                                lhsT=st[:, kc % 2, l2, :],
                                rhs=w[:, l2, kc, :],
                                start=(i == 0), stop=(i == L * KC - 1),
                            )
                if ly == 0:
                    # rest of the fp16 residents (layer-2 only) + MLP
                    # weights: queue them behind layer 1's e3m4 stream
                    for l in range(L8_LO, R_RES):
                        nc.sync.dma_start(wres[:, l], wT_e[:, l])
                    mlpw_sb = cpool.tile([128, 2, KC, D], f16, tag="mlpw")
                    nc.sync.dma_start(mlpw_sb[:], mlpw_e)
                # relu(msg * recip) -> next h (fp16), chunked per kc
                # (all on vector: engines serialize on the shared pm bank)
                for kc in range(KC):
                    sl = slice(kc * 128, (kc + 1) * 128)
                    nc.vector.tensor_scalar(h[ly + 1][:, sl], pm[:, sl],
                                            recip[:], 0.0,
                                            Alu.mult, Alu.max)

            # -------- MLP ---------------------------------------------------
            w0T_v = mlpw_sb[:, 0]
            w1T_v = mlpw_sb[:, 1]
            h_own = h[NUM_LAYERS]
            hT = cpool.tile([128, KC, 128], f16, tag="hT")
            pt = mpsum.tile([128, KC, 128], f16, tag="mm", name="ptr")
            for kc in range(KC):
                nc.tensor.transpose(pt[:, kc, :],
                                    h_own[:, kc * 128:(kc + 1) * 128],
                                    identity[:])
            nc.vector.tensor_copy(hT[:], pt[:])

            x1T = cpool.tile([128, KC, 128], f16, tag="x1T")
            px1 = mpsum.tile([128, KC, 128], f32, tag="mm", name="px1")
            for blk in range(KC):
                for kc in range(KC):
                    nc.tensor.matmul(
                        px1[:, blk, :],
                        lhsT=w0T_v[:, kc, blk * 128:(blk + 1) * 128],
                        rhs=hT[:, kc, :],
                        start=(kc == 0), stop=(kc == KC - 1),
                    )
            for blk in range(KC):
                nc.vector.tensor_scalar(x1T[:, blk, :], px1[:, blk, :],
                                        b0_v[:, blk:blk + 1], 0.0,
                                        Alu.add, Alu.max)

            x2 = cpool.tile([128, KC, 128], f32, tag="x2")
            px2 = mpsum.tile([128, KC, 128], f32, tag="mm", name="px2")
            for blk in range(KC):
                for kc in range(KC):
                    nc.tensor.matmul(
                        px2[:, blk, :],
                        lhsT=w1T_v[:, kc, blk * 128:(blk + 1) * 128],
                        rhs=x1T[:, kc, :],
                        start=(kc == 0), stop=(kc == KC - 1),
                    )
            for blk in range(KC):
                nc.vector.tensor_scalar(x2[:, blk, :], px2[:, blk, :],
                                        b1_v[:, blk:blk + 1], 0.0,
                                        Alu.add, Alu.max)

            nc.sync.dma_start(out_e, x2[:])

    nc.compile()
    return nc


def _get_nc():
    if "nc" not in _CACHE:
        _CACHE["nc"] = _build_nc()
    return _CACHE["nc"]


def kernel(gcn_inputs, word_seq_len, adj_matrix, dep_label_matrix,
           w_params, mlp_w0, mlp_b0, mlp_w1, mlp_b1, **_unused):
    from concourse.bass_utils import run_bass_kernel_spmd

    gcn = np.asarray(gcn_inputs, dtype=np.float32)
    adj = np.asarray(adj_matrix, dtype=np.float32)
    lab = np.asarray(dep_label_matrix)
    w = np.asarray(w_params, dtype=np.float32)
    w0 = np.asarray(mlp_w0, dtype=np.float32)
    w1 = np.asarray(mlp_w1, dtype=np.float32)
    b0 = np.asarray(mlp_b0, dtype=np.float32)
    b1 = np.asarray(mlp_b1, dtype=np.float32)

    import ml_dtypes

    # wT[kmod, l, kc, d] = w[l, d, kc*128+kmod]  (shared by all cores)
    wT32 = w.transpose(0, 2, 1).reshape(L, KC, 128, D).transpose(2, 0, 1, 3)
    wT32 = np.ascontiguousarray(wT32)
    wT = wT32.astype(np.float16)
    # layer-1 e3m4 copy of labels L8_LO.., scaled x16 to clear denormals
    wT8 = np.ascontiguousarray(
        (wT32[:, L8_LO:] * W8_SCALE)).astype(ml_dtypes.float8_e3m4)
    w0T = w0.T.reshape(KC, 128, D).transpose(1, 0, 2)
    w1T = w1.T.reshape(KC, 128, D).transpose(1, 0, 2)
    mlpw = np.ascontiguousarray(
        np.stack([w0T, w1T], axis=1)).astype(np.float16)   # [128, 2, KC, D]
    b0r = b0.reshape(KC, 128).T                            # [128, KC]
    b1r = b1.reshape(KC, 128).T
    labf = lab.astype(np.float32)

    in_maps = []
    for c in range(NCORES):
        miscc = np.empty((N, N + 2 * KC), dtype=np.float32)
        miscc[:, 0:N] = adj[c]
        miscc[:, N:N + KC] = b0r
        miscc[:, N + KC:N + 2 * KC] = b1r
        in_maps.append({
            "gcn": gcn[c],
            "adjT": np.ascontiguousarray(adj[c].T),
            "labT": np.ascontiguousarray(labf[c].T),
            "misc": miscc,
            "wT": wT,
            "wT8": wT8,
            "mlpw": mlpw,
        })

    nc = _get_nc()
    res = run_bass_kernel_spmd(nc, in_maps, list(range(NCORES)))

    out = np.empty((B, N, D), dtype=np.float32)
    for c in range(NCORES):
        arr = res.results[c]["out"]          # [dmod, dblk, i]
        out[c] = np.transpose(arr, (2, 1, 0)).reshape(N, D)
    return out


# revision 37
# speedup vs baseline: 1.1560x; 1.0119x over previous
"""DepLabeledGCN Trainium2 kernel — data-parallel variant (no collectives).

Each core processes ITS OWN batch with ALL 48 label matrices:
    s-phase:  sT[kc,l] chunks = per-label masked-adjacency matmuls (fp16,
              masks exact 0/1), label PAIRS fused into N=256 matmuls
    msum:     msg = sum_{l,kc} sT[kc,l] @ W_l^T[kc], 192 accumulating
              matmuls into one PSUM bank per layer
    relu(msg * 1/denom) -> next layer h (chunked DVE/Act ops)
then the 2-layer MLP (PE-transpose + packed PSUM) on the same core.

Weights: 24 MB fp16 streamed per label from HBM on ONE hw queue (per-core
DMA is ~410 GB/s aggregate; more queues only delays the early pairs).
The first R_RES labels stay SBUF-resident for layer 2.

Scheduling details (measured on hw traces):
  - sT tile keeps the PSUM layout [q,kc,l,i]; the psum->sbuf cast is two
    contiguous halves on vector + scalar concurrently (gpsimd cannot
    access PSUM).  msum runs l2-major so each matmul only depends on
    one label's weight DMA (layer 1 is DMA-starved; finer deps matter).
  - weight DMAs stay per-label for the same reason.
  - h0 cast and the layer-boundary relu are chunked per kc to shorten
    the critical path into each layer's first matmuls.
"""

import sys

if '/opt/trn_rl_repo' not in sys.path:
    sys.path.insert(0, '/opt/trn_rl_repo')

import numpy as np

B, N, D, L = 8, 128, 512, 48
NCORES = 8
KC = D // 128
NUM_LAYERS = 2
R_RES = 28              # labels kept resident (fp16) for layer 2
NP = L // 2             # label pairs per layer
L8_LO = 4               # layer-1 labels >= L8_LO stream as e3m4 (x16)
L2_8LO = 36             # layer-2 labels >= L2_8LO stream as e3m4 (x16)
W8_SCALE = 16.0


def _use8(ly, p):
    return (ly == 0 and 2 * p >= L8_LO) or (ly == 1 and 2 * p >= L2_8LO)

_CACHE = {}


def _build_nc():
    import concourse.bass as bass
    import concourse.mybir as mybir
    import concourse.tile as tile
    from concourse import bacc
    from concourse.masks import make_identity

    dt = mybir.dt
    f32 = dt.float32
    f16 = dt.float16
    Alu = mybir.AluOpType
    Act = mybir.ActivationFunctionType

    nc = bacc.Bacc("TRN2", target_bir_lowering=False, debug=False,
                   num_devices=NCORES)

    gcn_e = nc.dram_tensor("gcn", [N, D], f32, kind="ExternalInput").ap()
    adjT_e = nc.dram_tensor("adjT", [N, N], f32, kind="ExternalInput").ap()
    labT_e = nc.dram_tensor("labT", [N, N], f32, kind="ExternalInput").ap()
    # misc: adjR (row-major adj) + b0 + b1 packed
    misc_e = nc.dram_tensor("misc", [N, N + 2 * KC], f32,
                            kind="ExternalInput").ap()
    wT_e = nc.dram_tensor("wT", [128, L, KC, D], f16, kind="ExternalInput").ap()
    # layer-1 copy of labels L8_LO..L-1, e3m4 scaled x16 (half the DMA
    # bytes; the 1/16 is folded into those pairs' sT casts)
    wT8_e = nc.dram_tensor("wT8", [128, L - L8_LO, KC, D], dt.float8e3,
                           kind="ExternalInput").ap()
    mlpw_e = nc.dram_tensor("mlpw", [128, 2, KC, D], f16,
                            kind="ExternalInput").ap()
    out_e = nc.dram_tensor("out", [128, KC, 128], f32,
                           kind="ExternalOutput").ap()

    with tile.TileContext(nc) as tc:
        with (
            tc.tile_pool(name="const", bufs=1) as cpool,
            tc.tile_pool(name="sTa", bufs=4) as sTa_pool,
            tc.tile_pool(name="sTb", bufs=4) as sTb_pool,
            tc.tile_pool(name="wst", bufs=4) as wst_pool,
            tc.tile_pool(name="w8", bufs=4) as w8_pool,
            tc.tile_pool(name="spsa", bufs=3, space="PSUM") as spsa,
            tc.tile_pool(name="spsb", bufs=3, space="PSUM") as spsb,
            tc.tile_pool(name="mpsum", bufs=2, space="PSUM") as mpsum,
        ):
            # -------- critical-path input loads -----------------------------
            adjT_sb = cpool.tile([128, N], f32, tag="adjT")
            nc.sync.dma_start(adjT_sb[:], adjT_e)
            labT_sb = cpool.tile([128, N], f32, tag="labT")
            nc.sync.dma_start(labT_sb[:], labT_e)
            gcn_sb = cpool.tile([128, D], f32, tag="gcn_sb")
            nc.sync.dma_start(gcn_sb[:], gcn_e)

            h = [cpool.tile([128, D], f16, tag=f"h{ly}", name=f"h{ly}")
                 for ly in range(NUM_LAYERS + 1)]
            nc.scalar.copy(h[0][:], gcn_sb[:])

            # resident fp16 weights. Labels 0..L8_LO-1 load now (layer 1
            # consumes them JIT); labels L8_LO..R_RES-1 are only needed by
            # layer 2 and load after layer 1's e3m4 stream (queue is FIFO).
            wres = cpool.tile([128, R_RES, KC, D], f16, tag="wres")
            for l in range(L8_LO):
                nc.sync.dma_start(wres[:, l], wT_e[:, l])

            # -------- masks: maskT[j, l, i] = (labT == l) * adjT ------------
            # pairs 0..5 upfront; the rest interleaved into the layer-1 loop
            maskT = cpool.tile([128, L, N], f16, tag="maskT")

            def emit_mask(l):
                nc.vector.scalar_tensor_tensor(
                    out=maskT[:, l, :],
                    in0=labT_sb[:],
                    scalar=float(l),
                    in1=adjT_sb[:],
                    op0=Alu.is_equal,
                    op1=Alu.mult,
                )

            for l in range(12):
                emit_mask(l)

            misc_sb = cpool.tile([128, N + 2 * KC], f32, tag="misc")
            nc.sync.dma_start(misc_sb[:], misc_e)
            adjR_v = misc_sb[:, 0:N]
            b0_v = misc_sb[:, N:N + KC]
            b1_v = misc_sb[:, N + KC:N + 2 * KC]

            den = cpool.tile([128, 1], f32, tag="den")
            nc.vector.tensor_reduce(den[:], adjR_v, mybir.AxisListType.X,
                                    Alu.add)
            nc.vector.tensor_scalar_add(den[:], den[:], 1.0)
            recip = cpool.tile([128, 1], f32, tag="recip")
            nc.vector.reciprocal(recip[:], den[:])

            # identity for the MLP transposes (gpsimd, idle at start)
            identity = cpool.tile([128, 128], f16, tag="ident")
            make_identity(nc, identity[:])

            # -------- GCN layers --------------------------------------------
            def emit_s(ly, p):
                """s-phase for label pair p: one N=256 matmul per kc.
                kc 0/1 and kc 2/3 use SEPARATE psum+sbuf tiles so the
                vector and scalar psum->sbuf casts run truly in parallel
                (engines serialize on a shared psum tile).  For layer-1
                pairs whose weights stream as e3m4 (x16), the cast applies
                the exact 1/16 compensation."""
                ps_a = spsa.tile([128, 2, 2, 128], f32, tag="spsa",
                                 name="spsa")
                ps_b = spsb.tile([128, 2, 2, 128], f32, tag="spsb",
                                 name="spsb")
                for kc in range(KC):
                    ps = ps_a if kc < 2 else ps_b
                    nc.tensor.matmul(
                        ps[:, kc % 2, :, :],
                        lhsT=h[ly][:, kc * 128:(kc + 1) * 128],
                        rhs=maskT[:, 2 * p:2 * p + 2, :],
                        start=True, stop=True,
                    )
                sa = sTa_pool.tile([128, 2, 2, 128], f16, tag="sTa",
                                   name="sTa")
                sb = sTb_pool.tile([128, 2, 2, 128], f16, tag="sTb",
                                   name="sTb")
                if _use8(ly, p):
                    nc.vector.tensor_scalar_mul(sa[:], ps_a[:],
                                                1.0 / W8_SCALE)
                    nc.scalar.activation(sb[:], ps_b[:], Act.Copy,
                                         scale=1.0 / W8_SCALE)
                else:
                    nc.vector.tensor_copy(sa[:], ps_a[:])
                    nc.scalar.copy(sb[:], ps_b[:])
                return sa, sb

            def get_w(ly, p):
                """Weight pair p.  Layer 1: resident fp16 for labels
                < L8_LO, else an e3m4 stream tile.  Layer 2: resident fp16
                or a re-streamed fp16 tile.  Per-label DMAs keep the msum
                deps fine-grained."""
                if _use8(ly, p):
                    w = w8_pool.tile([128, 2, KC, D], dt.float8e3, tag="w8",
                                     name="w8")
                    nc.sync.dma_start(w[:, 0], wT8_e[:, 2 * p - L8_LO])
                    nc.sync.dma_start(w[:, 1], wT8_e[:, 2 * p + 1 - L8_LO])
                    return w
                if 2 * p + 1 < R_RES:
                    return wres[:, 2 * p:2 * p + 2]
                w = wst_pool.tile([128, 2, KC, D], f16, tag="wst", name="wst")
                nc.sync.dma_start(w[:, 0], wT_e[:, 2 * p])
                nc.sync.dma_start(w[:, 1], wT_e[:, 2 * p + 1])
                return w

            S_AHEAD = 2
            for ly in range(NUM_LAYERS):
                pm = mpsum.tile([128, D], f32, tag="mm", name="mm")
                sT_q = [emit_s(ly, q) for q in range(S_AHEAD)]
                for p in range(NP):
                    if ly == 0 and 2 * (p + 6) < L:
                        emit_mask(2 * (p + 6))
                        emit_mask(2 * (p + 6) + 1)
                    if p + S_AHEAD < NP:
                        sT_q.append(emit_s(ly, p + S_AHEAD))
                    w = get_w(ly, p)
                    sa, sb = sT_q[p]
                    for l2 in range(2):
                        for kc in range(KC):
                            i = (p * 2 + l2) * KC + kc
                            st = sa if kc < 2 else sb
                            nc.tensor.matmul(
                                pm[:],
                                lhsT=st[:, kc % 2, l2, :],
                                rhs=w[:, l2, kc, :],
                                start=(i == 0), stop=(i == L * KC - 1),
                            )
                if ly == 0:
                    # rest of the fp16 residents (layer-2 only) + MLP
                    # weights: queue them behind layer 1's e3m4 stream
                    for l in range(L8_LO, R_RES):
                        nc.sync.dma_start(wres[:, l], wT_e[:, l])
                    mlpw_sb = cpool.tile([128, 2, KC, D], f16, tag="mlpw")
                    nc.sync.dma_start(mlpw_sb[:], mlpw_e)
                # relu(msg * recip) -> next h (fp16), chunked per kc
                # (all on vector: engines serialize on the shared pm bank)
                for kc in range(KC):
                    sl = slice(kc * 128, (kc + 1) * 128)
                    nc.vector.tensor_scalar(h[ly + 1][:, sl], pm[:, sl],
                                            recip[:], 0.0,
                                            Alu.mult, Alu.max)

            # -------- MLP ---------------------------------------------------
            w0T_v = mlpw_sb[:, 0]
            w1T_v = mlpw_sb[:, 1]
            h_own = h[NUM_LAYERS]
            hT = cpool.tile([128, KC, 128], f16, tag="hT")
            pt = mpsum.tile([128, KC, 128], f16, tag="mm", name="ptr")
            for kc in range(KC):
                nc.tensor.transpose(pt[:, kc, :],
                                    h_own[:, kc * 128:(kc + 1) * 128],
                                    identity[:])
            nc.vector.tensor_copy(hT[:], pt[:])

            x1T = cpool.tile([128, KC, 128], f16, tag="x1T")
            px1 = mpsum.tile([128, KC, 128], f32, tag="mm", name="px1")
            for blk in range(KC):
                for kc in range(KC):
                    nc.tensor.matmul(
                        px1[:, blk, :],
                        lhsT=w0T_v[:, kc, blk * 128:(blk + 1) * 128],
                        rhs=hT[:, kc, :],
                        start=(kc == 0), stop=(kc == KC - 1),
                    )
            for blk in range(KC):
                nc.vector.tensor_scalar(x1T[:, blk, :], px1[:, blk, :],
                                        b0_v[:, blk:blk + 1], 0.0,
                                        Alu.add, Alu.max)

            x2 = cpool.tile([128, KC, 128], f32, tag="x2")
            px2 = mpsum.tile([128, KC, 128], f32, tag="mm", name="px2")
            for blk in range(KC):
                for kc in range(KC):
                    nc.tensor.matmul(
                        px2[:, blk, :],
                        lhsT=w1T_v[:, kc, blk * 128:(blk + 1) * 128],
                        rhs=x1T[:, kc, :],
                        start=(kc == 0), stop=(kc == KC - 1),
                    )
            for blk in range(KC):
                nc.vector.tensor_scalar(x2[:, blk, :], px2[:, blk, :],
                                        b1_v[:, blk:blk + 1], 0.0,
                                        Alu.add, Alu.max)

            nc.sync.dma_start(out_e, x2[:])

    nc.compile()
    return nc


def _get_nc():
    if "nc" not in _CACHE:
        _CACHE["nc"] = _build_nc()
    return _CACHE["nc"]


def kernel(gcn_inputs, word_seq_len, adj_matrix, dep_label_matrix,
           w_params, mlp_w0, mlp_b0, mlp_w1, mlp_b1, **_unused):
    from concourse.bass_utils import run_bass_kernel_spmd

    gcn = np.asarray(gcn_inputs, dtype=np.float32)
    adj = np.asarray(adj_matrix, dtype=np.float32)
    lab = np.asarray(dep_label_matrix)
    w = np.asarray(w_params, dtype=np.float32)
    w0 = np.asarray(mlp_w0, dtype=np.float32)
    w1 = np.asarray(mlp_w1, dtype=np.float32)
    b0 = np.asarray(mlp_b0, dtype=np.float32)
    b1 = np.asarray(mlp_b1, dtype=np.float32)

    import ml_dtypes

    # wT[kmod, l, kc, d] = w[l, d, kc*128+kmod]  (shared by all cores)
    wT32 = w.transpose(0, 2, 1).reshape(L, KC, 128, D).transpose(2, 0, 1, 3)
    wT32 = np.ascontiguousarray(wT32)
    wT = wT32.astype(np.float16)
    # layer-1 e3m4 copy of labels L8_LO.., scaled x16 to clear denormals
    wT8 = np.ascontiguousarray(
        (wT32[:, L8_LO:] * W8_SCALE)).astype(ml_dtypes.float8_e3m4)
    w0T = w0.T.reshape(KC, 128, D).transpose(1, 0, 2)
    w1T = w1.T.reshape(KC, 128, D).transpose(1, 0, 2)
    mlpw = np.ascontiguousarray(
        np.stack([w0T, w1T], axis=1)).astype(np.float16)   # [128, 2, KC, D]
    b0r = b0.reshape(KC, 128).T                            # [128, KC]
    b1r = b1.reshape(KC, 128).T
    labf = lab.astype(np.float32)

    in_maps = []
    for c in range(NCORES):
        miscc = np.empty((N, N + 2 * KC), dtype=np.float32)
        miscc[:, 0:N] = adj[c]
        miscc[:, N:N + KC] = b0r
        miscc[:, N + KC:N + 2 * KC] = b1r
        in_maps.append({
            "gcn": gcn[c],
            "adjT": np.ascontiguousarray(adj[c].T),
            "labT": np.ascontiguousarray(labf[c].T),
            "misc": miscc,
            "wT": wT,
            "wT8": wT8,
            "mlpw": mlpw,
        })

    nc = _get_nc()
    res = run_bass_kernel_spmd(nc, in_maps, list(range(NCORES)))

    out = np.empty((B, N, D), dtype=np.float32)
    for c in range(NCORES):
        arr = res.results[c]["out"]          # [dmod, dblk, i]
        out[c] = np.transpose(arr, (2, 1, 0)).reshape(N, D)
    return out
